# Initial kernel scaffold
#
"""Cascaded-attention GRU recurrence on 8 NeuronCores (Bass/Tile).

Problem: B=128, T=75, D=512, V=28. Data-parallel over batch: 16 batch rows
per core, weights replicated. Per-core recurrence over 75 steps with two
independent batch half-groups (8 rows each) pipelined against each other.

Key layout choices (per core, BL=16 local batch):
- d-on-partition layout for the big tensors: UaH' = x@Ua + Ba1 + Ba2 stored
  [128(d-chunk), 4(chunk), 16(b), 75(t)]; per-step tanh runs as one ACT
  instruction per half-group.
- WaS bias (state@Wa, changes per step) broadcast over t via a stride-0 AP
  on the DVE tensor_tensor add.
- scores contracted against Va via PE with a column-replicated Va (VaSEL) so
  the result lands partition-major-ish as REP[75, (b,t)] (all rows equal);
  the free->partition transpose of the softmax weights is done by masking
  REP with a constant diagonal and free-reducing (DVE), yielding
  scoresT[75(t), b].
- softmax normalization deferred: unnormalized exp(scoresT) drives
  block-diag matmuls against XKC = x@[gru_kernel|Co] (precomputed on
  device), producing xm/CoC directly ([8, 112] per group); 1/Z folded into
  the GRU gate/output scalar ops.
- sigmoid via tanh (same ACT table set as exp): sigmoid(a) = .5 + .5*tanh(a/2),
  with gru_rec_kernel pre-halved on host so gate args come out right.
- embedding lookup: softmax probs cast to int32 are 0 unless pred == 1.0, so
  emb[idx]@Wo == w0 + (w1-w0)*[pred >= 1], w = emb@Wo (exact).
- Ba3 dropped (softmax shift invariance). gru_bias[1] folded into the hm
  matmul; gru_bias[0] assumed zero (asserted) as in the problem setup.
"""

import numpy as np

B, T, D, V = 128, 75, 512, 28
NCORES = 8
BL = B // NCORES        # 16 batch rows per core
G = 2                   # half-groups per core
BG = BL // G            # 8 rows per group
SUBS = [(0, 6), (6, 6), (12, 4)]  # softmax sub-groups (psum bank = 512 f32)
NC_, CH = 128, D // 128  # partitions, d-chunks
USE_GP_BIAS = False
USE_GP_SMB = False
USE_TMODE = False  # is_transpose matmuls: 4x faster fp32 weight path


def _build(nc, tc, tile, bass, mybir, gru_b0_nonzero, steps=T):
    f32 = mybir.dt.float32
    Act = mybir.ActivationFunctionType
    Op = mybir.AluOpType

    # ---------------- DRAM I/O ----------------
    dr = {}
    def din(name, shape):
        dr[name] = nc.dram_tensor(name, shape, f32, kind="ExternalInput")
        return dr[name]

    x_dmaj = din("x_dmaj", [NC_, CH, BL, T])
    ua_k = din("ua_k", [NC_, CH, CH, 128])
    ba12 = din("ba12", [NC_, CH])
    wa = din("wa", [V, D])
    vasel = din("vasel", [NC_, CH, T])
    w2 = din("w2", [NC_, CH, 112])
    wrec_h = din("wrec_h", [V + 1, 84])    # [0.5*gru_rec_kernel; 0.5*gru_bias1]
    uo = din("uo", [V + 1, V])             # [Uo; Bo + w0]
    diag = din("diag", [T, T])
    i16 = din("i16", [BL, BL])
    onesrow = din("onesrow", [1, BL])
    twos75 = din("twos75", [T, 1])
    dwrep = din("dwrep", [BG, V])          # w1-w0, replicated
    if gru_b0_nonzero:
        b0rep = din("b0rep", [BG, 84])
    y_out = nc.dram_tensor("y", [BL, T, V], f32, kind="ExternalOutput")

    import contextlib
    ctx = contextlib.ExitStack()
    with ctx:
        cst = ctx.enter_context(tc.tile_pool(name="cst", bufs=1))
        wrk = ctx.enter_context(tc.tile_pool(name="wrk", bufs=2))
        wrk3 = ctx.enter_context(tc.tile_pool(name="wrk3", bufs=3))
        pwast = ctx.enter_context(tc.tile_pool(name="pwast", bufs=1, space="PSUM"))
        pbt = ctx.enter_context(tc.tile_pool(name="pbt", bufs=2, space="PSUM"))
        prep = ctx.enter_context(tc.tile_pool(name="prep", bufs=3, space="PSUM"))
        pxm = ctx.enter_context(tc.tile_pool(name="pxm", bufs=2, space="PSUM"))

        # ---------------- constants into SBUF ----------------
        t_x = cst.tile([NC_, CH, BL, T], f32, tag="t_x")
        t_ua = cst.tile([NC_, CH, CH, 128], f32, tag="t_ua")
        t_ba12 = cst.tile([NC_, CH], f32, tag="t_ba12")
        t_wa = cst.tile([V, D], f32, tag="t_wa")
        t_vas = cst.tile([NC_, CH, T], f32, tag="t_vas")
        t_w2 = cst.tile([NC_, CH, 112], f32, tag="t_w2")
        t_wrec = cst.tile([V + 1, 84], f32, tag="t_wrec")
        t_uo = cst.tile([V + 1, V], f32, tag="t_uo")
        t_diag = cst.tile([T, T], f32, tag="t_diag")
        t_i16 = cst.tile([BL, BL], f32, tag="t_i16")
        t_two = cst.tile([T, 1], f32, tag="t_two")
        nc.sync.dma_start(t_two[:], twos75[:])
        t_dw = cst.tile([BG, V], f32, tag="t_dw")
        for tt, d_ in [(t_x, x_dmaj), (t_ua, ua_k), (t_ba12, ba12), (t_wa, wa),
                       (t_vas, vasel), (t_w2, w2), (t_wrec, wrec_h),
                       (t_uo, uo), (t_diag, diag),
                       (t_i16, i16), (t_dw, dwrep)]:
            nc.sync.dma_start(tt[:], d_[:])
        if gru_b0_nonzero:
            t_b0 = cst.tile([BG, 84], f32, tag="t_b0")
            nc.sync.dma_start(t_b0[:], b0rep[:])

        # persistent state/work tiles
        t_uahp = cst.tile([NC_, CH, BL, T], f32, tag="t_uahp")     # x@Ua + Ba1 + Ba2
        t_xkc = cst.tile([T, BL, 113], f32, tag="t_xkc")           # x@[gruK|Co|1]
        t_smb = cst.tile([T, BL * BL], f32, tag="t_smb")           # block-diag exp(scoresT)
        t_out = []
        for g in range(G):
            t_out_g = cst.tile([BG, T, V], f32, tag=f"t_out{g}")
            if steps < T:  # truncated builds (timing/sim only): avoid
                nc.vector.memset(t_out_g[:], 0.0)  # uninit reads at final DMA
            t_out.append(t_out_g)
        nc.vector.memset(t_smb[:], 0.0)

        # ---------------- preamble: UaH' = x@Ua + (Ba1+Ba2) ----------------
        NSL, SL = 3, 400  # bt slices per e-chunk
        for ec in range(CH):
            uah_flat = t_uahp[:, ec, :, :].rearrange("p b t -> p (b t)")
            for i in range(NSL):
                ps = prep.tile([NC_, SL], f32, tag="prep")
                for dc in range(CH):
                    x_sl = t_x[:, dc, :, :].rearrange("p b t -> p (b t)")[
                        :, i * SL:(i + 1) * SL]
                    nc.tensor.matmul(ps[:], t_ua[:, dc, ec, :], x_sl,
                                     start=(dc == 0), stop=(dc == CH - 1))
                nc.scalar.activation(uah_flat[:, i * SL:(i + 1) * SL], ps[:],
                                     Act.Identity, bias=t_ba12[:, ec:ec + 1],
                                     scale=1.0)

        # ---------------- preamble: XKC = x@[gruK|Co], ones col ----------------
        for b in range(BL):
            ps = pxm.tile([T, 112], f32, tag="pxm")
            for dc in range(CH):
                nc.tensor.matmul(ps[:], t_x[:, dc, b, :], t_w2[:, dc, :],
                                 start=(dc == 0), stop=(dc == CH - 1))
            nc.vector.tensor_copy(t_xkc[:, b, 0:112], ps[:])
        ones_col = bass.AP(t_xkc.tensor, t_xkc[:].offset + 112,
                           [list(t_xkc[:].ap[0]), [113, BL]])
        nc.vector.memset(ones_col, 1.0)

        # ---------------- recurrent state ----------------
        state = []   # [8,28] b-major per group
        for g in range(G):
            sg = wrk.tile([BG, V], f32, tag=f"state{g}")
            nc.vector.memset(sg[:], 0.0)
            state.append(sg)
        stateT = wrk.tile([V + 1, BL], f32, tag="stateT")  # shared; row V = ones
        nc.vector.memset(stateT[0:V, :], 0.0)
        nc.sync.dma_start(stateT[V:V + 1, :], onesrow[:])

        # ---------------- the 75 steps ----------------
        has_gp_tt = hasattr(nc.gpsimd, "tensor_tensor")
        for s in range(steps):
            new_state = [None] * G
            bt_list = [None] * G
            gorder = (0, 1)
            tanhY = wrk.tile([NC_, CH, BL, T], f32, tag="tanhY")
            # --- phase 0 (shared): WaS^T for both groups ---
            wast_ps = pwast.tile([NC_, 72], f32, tag="wastps")
            if s > 0:
                for c in range(CH):
                    nc.tensor.matmul(wast_ps[:, c * 16:(c + 1) * 16],
                                     t_wa[:, c * 128:(c + 1) * 128],
                                     stateT[0:V, :], start=True, stop=True,
                                     is_transpose=USE_TMODE)
                if has_gp_tt and USE_GP_BIAS:
                    # SBUF copy of the c2/c3 half for GPSIMD (no PSUM there)
                    wast_sb = wrk.tile([NC_, 32], f32, tag="wastsb")
                    nc.vector.tensor_copy(wast_sb[:], wast_ps[:, 32:64])
            # --- phase 1 (per group): state matmuls, bias-add, tanh ---
            for g in gorder:
                sg = state[g]
                sTg = stateT[:, g * BG:(g + 1) * BG]  # [29, 8]; row 28 = 1
                bs = g * BG
                bt_ps = pbt.tile([BG, 256], f32, tag="btps")
                bt_list[g] = bt_ps
                nc.tensor.matmul(bt_ps[:, 0:84], sTg, t_wrec[:],
                                 start=True, stop=True)
                nc.tensor.matmul(bt_ps[:, 84:112], sTg, t_uo[:],
                                 start=True, stop=True)

                ty_out = tanhY[:, :, bs:bs + BG, :]
                if s > 0:
                    Yg = wrk.tile([NC_, CH, BG, T], f32, tag=f"Y{g}")

                    def bias_add(eng, src, off, c0, cn):
                        # WaS^T broadcast over t (src: PSUM for DVE, SBUF copy
                        # for GPSIMD which cannot access PSUM)
                        w_sl = src[:, off + bs:off + bs + 8]
                        w_bc = bass.AP(
                            w_sl.tensor, w_sl.offset,
                            [list(w_sl.ap[0]), [16, cn], [1, 8], [0, T]])
                        eng.tensor_tensor(Yg[:, c0:c0 + cn, :, :],
                                          t_uahp[:, c0:c0 + cn, bs:bs + BG, :],
                                          w_bc, Op.add)
                    # split chunk-pairs so tanh(c01) overlaps bias-add(c23);
                    # c23 on GPSIMD runs concurrently with DVE's c01
                    if has_gp_tt and USE_GP_BIAS:
                        bias_add(nc.gpsimd, wast_sb, 0, 2, 2)
                        bias_add(nc.vector, wast_ps, 0, 0, 2)
                    else:
                        bias_add(nc.vector, wast_ps, 0, 0, 2)
                        bias_add(nc.vector, wast_ps, 32, 2, 2)
                    nc.scalar.activation(tanhY[:, 0:2, bs:bs + BG, :],
                                         Yg[:, 0:2, :, :], Act.Tanh)
                    nc.scalar.activation(tanhY[:, 2:4, bs:bs + BG, :],
                                         Yg[:, 2:4, :, :], Act.Tanh)
                else:
                    nc.scalar.activation(ty_out, t_uahp[:, :, bs:bs + BG, :],
                                         Act.Tanh)

            # --- phase 2 (shared, sub-granular): scoresT -> exp -> SmBlk ->
            # xm/CoC chunks emitted per sub-group so group tails start early
            scT = wrk.tile([T, BL], f32, tag="scT")
            expT = wrk.tile([T, BL], f32, tag="expT")
            xm_list = [pxm.tile([BG, 113], f32, tag="pxm", name=f"xm{g}_{s}")
                       for g in range(G)]
            for b0, nb in SUBS:
                rep_ps = prep.tile([T, 6 * T], f32, tag="prep")
                rep = rep_ps[:].rearrange("p (b t) -> p b t", b=6)[:, 0:nb, :]
                for c in range(CH):
                    nc.tensor.matmul(rep, t_vas[:, c, :],
                                     tanhY[:, c, b0:b0 + nb, :],
                                     start=(c == 0), stop=(c == CH - 1),
                                     is_transpose=USE_TMODE)
                msk = wrk3.tile([T, 6, T], f32, tag="msk")
                d_ap = t_diag[:]
                d_bc = bass.AP(d_ap.tensor, d_ap.offset,
                               [list(d_ap.ap[0]), [0, nb], list(d_ap.ap[1])])
                nc.vector.tensor_tensor(msk[:, 0:nb, :], rep, d_bc, Op.mult)
                nc.vector.tensor_reduce(scT[:, b0:b0 + nb], msk[:, 0:nb, :],
                                        mybir.AxisListType.X, Op.add)
                nc.scalar.activation(expT[:, b0:b0 + nb], scT[:, b0:b0 + nb],
                                     Act.Exp)
                smb_dst = bass.AP(t_smb.tensor, t_smb[:].offset + 17 * b0,
                                  [list(t_smb[:].ap[0]), [17, nb]])
                nc.vector.tensor_copy(smb_dst, expT[:, b0:b0 + nb])
                for b in range(b0, b0 + nb):
                    g = b // BG
                    bs = g * BG
                    nc.tensor.matmul(
                        xm_list[g][:],
                        t_smb[:, 16 * b + bs:16 * b + bs + BG],
                        t_xkc[:, b, :],
                        start=(b == bs), stop=(b == bs + BG - 1))
                for g in range(G):  # groups whose expT slice just completed
                    if b0 < (g + 1) * BG <= b0 + nb:
                        nc.tensor.matmul(bt_list[g][:, 112:113],
                                         expT[:, g * BG:(g + 1) * BG],
                                         t_two[:], start=True, stop=True)

            # --- phase 3 (per group): recip, gates, state, pred ---
            for g in gorder:
                sg = state[g]
                bs = g * BG
                bt_ps = bt_list[g]
                xm_ps = xm_list[g]
                hm_sb = wrk.tile([BG, 84], f32, tag=f"hm{g}")
                nc.vector.tensor_copy(hm_sb[:], bt_ps[:, 0:84])
                rhalf = wrk.tile([BG, 1], f32, tag=f"rhalf{g}")
                nc.vector.reciprocal(rhalf[:], bt_ps[:, 112:113])
                rfull = wrk.tile([BG, 1], f32, tag=f"rfull{g}")
                nc.vector.tensor_scalar(rfull[:], rhalf[:], 2.0, None, Op.mult)

                # --- GRU gates ---
                zr = wrk.tile([BG, 56], f32, tag=f"zr{g}")
                nc.vector.scalar_tensor_tensor(zr[:], xm_ps[:, 0:56], rhalf[:],
                                               hm_sb[:, 0:56], Op.mult, Op.add)
                if gru_b0_nonzero:
                    nc.vector.tensor_tensor(zr[:], zr[:], t_b0[:, 0:56], Op.add)
                tz = wrk.tile([BG, 56], f32, tag=f"tz{g}")
                nc.scalar.activation(tz[:], zr[:], Act.Tanh)
                s1 = wrk.tile([BG, V], f32, tag=f"s1{g}")
                nc.vector.scalar_tensor_tensor(s1[:], tz[:, V:56], 1.0,
                                               hm_sb[:, 56:84], Op.add, Op.mult)
                ah = wrk.tile([BG, V], f32, tag=f"ah{g}")
                nc.vector.scalar_tensor_tensor(ah[:], xm_ps[:, 56:84], rfull[:],
                                               s1[:], Op.mult, Op.add)
                if gru_b0_nonzero:
                    nc.vector.tensor_tensor(ah[:], ah[:], t_b0[:, 56:84], Op.add)
                hh = wrk.tile([BG, V], f32, tag=f"hh{g}")
                nc.scalar.activation(hh[:], ah[:], Act.Tanh)
                d1 = wrk.tile([BG, V], f32, tag=f"d1{g}")
                nc.vector.tensor_sub(d1[:], sg[:], hh[:])
                d2 = wrk.tile([BG, V], f32, tag=f"d2{g}")
                nc.vector.tensor_add(d2[:], sg[:], hh[:])
                m1 = wrk.tile([BG, V], f32, tag=f"m1{g}")
                nc.vector.tensor_mul(m1[:], tz[:, 0:V], d1[:])
                ns = wrk.tile([BG, V], f32, tag=f"state{g}")
                nc.vector.tensor_tensor(ns[:], m1[:], d2[:], Op.add)
                nc.vector.tensor_scalar(ns[:], ns[:], 0.5, None, Op.mult)

                # --- stateT for next step (into shared stateT col-slice) ---
                nc.tensor.transpose(wast_ps[0:V, 64:72], ns[:], t_i16[0:BG, 0:BG])
                nc.vector.tensor_copy(stateT[0:V, bs:bs + BG],
                                      wast_ps[0:V, 64:72])

                # --- pred logits + softmax (Bo+w0 folded into UoH psum) ---
                l1 = wrk.tile([BG, V], f32, tag=f"l1{g}")
                if s > 0:
                    l2 = wrk.tile([BG, V], f32, tag=f"l2{g}")
                    nc.vector.scalar_tensor_tensor(l2[:], t_out[g][:, s - 1, :],
                                                   1.0, t_dw[:], Op.is_ge,
                                                   Op.mult)
                    nc.vector.scalar_tensor_tensor(
                        l1[:], xm_ps[:, 84:112], rfull[:], l2[:],
                        Op.mult, Op.add)
                else:
                    nc.vector.tensor_scalar(l1[:], xm_ps[:, 84:112], rfull[:],
                                            None, Op.mult)
                logits = wrk.tile([BG, V], f32, tag=f"logits{g}")
                nc.vector.tensor_tensor(logits[:], l1[:], bt_ps[:, 84:112],
                                        Op.add)
                expP = wrk.tile([BG, V], f32, tag=f"expP{g}")
                zp = wrk.tile([BG, 1], f32, tag=f"zp{g}")
                nc.scalar.activation(expP[:], logits[:], Act.Exp,
                                     accum_out=zp[:])
                rp = wrk.tile([BG, 1], f32, tag=f"rp{g}")
                nc.vector.reciprocal(rp[:], zp[:])
                nc.vector.tensor_scalar(t_out[g][:, s, :], expP[:], rp[:],
                                        None, Op.mult)

                new_state[g] = ns

            state = new_state

        for g in range(G):
            nc.sync.dma_start(y_out[g * BG:(g + 1) * BG, :, :], t_out[g][:])
    return dr, y_out


_CACHE = {}


def _get_program(gru_b0_nonzero, steps=T):
    key = (bool(gru_b0_nonzero), steps)
    if key in _CACHE:
        return _CACHE[key]
    import concourse.bass as bass
    import concourse.bacc as bacc
    import concourse.tile as tile
    from concourse import mybir

    nc = bacc.Bacc("TRN2", target_bir_lowering=False, debug=False,
                   num_devices=NCORES)
    with tile.TileContext(nc) as tc:
        _build(nc, tc, tile, bass, mybir, gru_b0_nonzero, steps)
    nc.compile()
    _CACHE[key] = nc
    return nc


def _prep_core_inputs(inputs, core):
    x = inputs["x"]
    xs = np.ascontiguousarray(x[core * BL:(core + 1) * BL]).astype(np.float32)
    # [16,75,512] -> [128, chunk, b, t]
    x_dmaj = np.ascontiguousarray(
        xs.reshape(BL, T, CH, 128).transpose(3, 2, 0, 1))
    return x_dmaj


def _prep_weights(inputs):
    f = np.float32
    Ua = inputs["Ua"].astype(f)
    ua_k = np.ascontiguousarray(
        Ua.reshape(CH, 128, CH, 128).transpose(1, 0, 2, 3))
    ba = (inputs["Ba1"] + inputs["Ba2"]).astype(f).reshape(CH, 128)
    ba12 = np.ascontiguousarray(ba.T)
    Va = inputs["Va"].astype(f).reshape(CH, 128)
    vasel = np.ascontiguousarray(
        np.repeat(Va.T[:, :, None], T, axis=2))
    w2 = np.concatenate([inputs["gru_kernel"], inputs["Co"]], axis=1).astype(f)
    w2 = np.ascontiguousarray(w2.reshape(CH, 128, 112).transpose(1, 0, 2))
    w = (inputs["emb"].astype(f) @ inputs["Wo"].astype(f)).reshape(-1)
    w0, w1 = float(w[0]), float(w[1])
    gb = inputs["gru_bias"].astype(f)
    out = {
        "ua_k": ua_k, "ba12": ba12, "wa": inputs["Wa"].astype(f),
        "vasel": vasel, "w2": w2,
        "wrec_h": np.concatenate(
            [0.5 * inputs["gru_rec_kernel"].astype(f), 0.5 * gb[1:2]], axis=0),
        "uo": np.concatenate(
            [inputs["Uo"].astype(f), inputs["Bo"].astype(f) + w0], axis=0),
        "diag": np.eye(T, dtype=f),
        "i16": np.eye(BL, dtype=f),
        "onesrow": np.ones([1, BL], dtype=f),
        "twos75": np.full([T, 1], 2.0, dtype=f),
        "dwrep": np.full([BG, V], w1 - w0, dtype=f),
    }
    b0 = gb[0]
    if np.any(b0 != 0):
        out["b0rep"] = np.repeat(b0[None, :], BG, axis=0)
    return out, bool(np.any(b0 != 0))


def kernel(**inputs):
    from concourse.bass_utils import run_bass_kernel_spmd

    weights, b0nz = _prep_weights(inputs)
    nc = _get_program(b0nz)
    in_maps = []
    for core in range(NCORES):
        m = dict(weights)
        m["x_dmaj"] = _prep_core_inputs(inputs, core)
        in_maps.append(m)
    res = run_bass_kernel_spmd(nc, in_maps, core_ids=list(range(NCORES)))
    out = np.concatenate([res.results[c]["y"] for c in range(NCORES)], axis=0)
    return out.astype(np.float32)





# revision 1
# speedup vs baseline: 1.1757x; 1.1757x over previous
"""Cascaded-attention GRU recurrence on 8 NeuronCores (Bass/Tile).

Problem: B=128, T=75, D=512, V=28. Data-parallel over batch: 16 batch rows
per core, weights replicated. Per-core recurrence over 75 steps with two
independent batch half-groups (8 rows each) pipelined against each other.

Key layout choices (per core, BL=16 local batch):
- d-on-partition layout for the big tensors: UaH' = x@Ua + Ba1 + Ba2 stored
  [128(d-chunk), 4(chunk), 16(b), 75(t)]; per-step tanh runs as one ACT
  instruction per half-group.
- WaS bias (state@Wa, changes per step) broadcast over t via a stride-0 AP
  on the DVE tensor_tensor add.
- scores contracted against Va via PE with a column-replicated Va (VaSEL) so
  the result lands partition-major-ish as REP[75, (b,t)] (all rows equal);
  the free->partition transpose of the softmax weights is done by masking
  REP with a constant diagonal and free-reducing (DVE), yielding
  scoresT[75(t), b].
- softmax normalization deferred: unnormalized exp(scoresT) drives
  block-diag matmuls against XKC = x@[gru_kernel|Co] (precomputed on
  device), producing xm/CoC directly ([8, 112] per group); 1/Z folded into
  the GRU gate/output scalar ops.
- sigmoid via tanh (same ACT table set as exp): sigmoid(a) = .5 + .5*tanh(a/2),
  with gru_rec_kernel pre-halved on host so gate args come out right.
- embedding lookup: softmax probs cast to int32 are 0 unless pred == 1.0, so
  emb[idx]@Wo == w0 + (w1-w0)*[pred >= 1], w = emb@Wo (exact).
- Ba3 dropped (softmax shift invariance). gru_bias[1] folded into the hm
  matmul; gru_bias[0] assumed zero (asserted) as in the problem setup.
"""

import numpy as np

B, T, D, V = 128, 75, 512, 28
NCORES = 8
BL = B // NCORES        # 16 batch rows per core
G = 2                   # half-groups per core
BG = BL // G            # 8 rows per group
SUBS = [(0, 6), (6, 6), (12, 4)]  # softmax sub-groups (psum bank = 512 f32)
NC_, CH = 128, D // 128  # partitions, d-chunks
USE_GP_BIAS = False
USE_GP_SMB = False
USE_TMODE = False  # is_transpose matmuls: 4x faster fp32 weight path


def _build(nc, tc, tile, bass, mybir, gru_b0_nonzero, steps=T):
    f32 = mybir.dt.float32
    Act = mybir.ActivationFunctionType
    Op = mybir.AluOpType

    # ---------------- DRAM I/O ----------------
    dr = {}
    def din(name, shape):
        dr[name] = nc.dram_tensor(name, shape, f32, kind="ExternalInput")
        return dr[name]

    x_dmaj = din("x_dmaj", [NC_, CH, BL, T])
    ua_k = din("ua_k", [NC_, CH, CH, 128])
    ba12 = din("ba12", [NC_, CH])
    wa = din("wa", [V, D])
    vasel = din("vasel", [NC_, CH, T])
    w2 = din("w2", [NC_, CH, 112])
    wrec_h = din("wrec_h", [V + 1, 84])    # [0.5*gru_rec_kernel; 0.5*gru_bias1]
    uo = din("uo", [V + 1, V])             # [Uo; Bo + w0]
    diag = din("diag", [T, T])
    i16 = din("i16", [BL, BL])
    onesrow = din("onesrow", [1, BL])
    twos75 = din("twos75", [T, 1])
    dwrep = din("dwrep", [BG, V])          # w1-w0, replicated
    if gru_b0_nonzero:
        b0rep = din("b0rep", [BG, 84])
    y_out = nc.dram_tensor("y", [BL, T, V], f32, kind="ExternalOutput")

    import contextlib
    ctx = contextlib.ExitStack()
    with ctx:
        cst = ctx.enter_context(tc.tile_pool(name="cst", bufs=1))
        wrk = ctx.enter_context(tc.tile_pool(name="wrk", bufs=2))
        wrk3 = ctx.enter_context(tc.tile_pool(name="wrk3", bufs=3))
        pwast = ctx.enter_context(tc.tile_pool(name="pwast", bufs=1, space="PSUM"))
        pbt = ctx.enter_context(tc.tile_pool(name="pbt", bufs=2, space="PSUM"))
        prep = ctx.enter_context(tc.tile_pool(name="prep", bufs=3, space="PSUM"))
        pxm = ctx.enter_context(tc.tile_pool(name="pxm", bufs=2, space="PSUM"))

        # ---------------- constants into SBUF ----------------
        t_x = cst.tile([NC_, CH, BL, T], f32, tag="t_x")
        t_ua = cst.tile([NC_, CH, CH, 128], f32, tag="t_ua")
        t_ba12 = cst.tile([NC_, CH], f32, tag="t_ba12")
        t_wa = cst.tile([V, D], f32, tag="t_wa")
        t_vas = cst.tile([NC_, CH, T], f32, tag="t_vas")
        t_w2 = cst.tile([NC_, CH, 112], f32, tag="t_w2")
        t_wrec = cst.tile([V + 1, 84], f32, tag="t_wrec")
        t_uo = cst.tile([V + 1, V], f32, tag="t_uo")
        t_diag = cst.tile([T, T], f32, tag="t_diag")
        t_i16 = cst.tile([BL, BL], f32, tag="t_i16")
        t_two = cst.tile([T, 1], f32, tag="t_two")
        nc.sync.dma_start(t_two[:], twos75[:])
        t_dw = cst.tile([BG, V], f32, tag="t_dw")
        for tt, d_ in [(t_x, x_dmaj), (t_ua, ua_k), (t_ba12, ba12), (t_wa, wa),
                       (t_vas, vasel), (t_w2, w2), (t_wrec, wrec_h),
                       (t_uo, uo), (t_diag, diag),
                       (t_i16, i16), (t_dw, dwrep)]:
            nc.sync.dma_start(tt[:], d_[:])
        if gru_b0_nonzero:
            t_b0 = cst.tile([BG, 84], f32, tag="t_b0")
            nc.sync.dma_start(t_b0[:], b0rep[:])

        # persistent state/work tiles
        t_uahp = cst.tile([NC_, CH, BL, T], f32, tag="t_uahp")     # x@Ua + Ba1 + Ba2
        t_xkc = cst.tile([T, BL, 113], f32, tag="t_xkc")           # x@[gruK|Co|1]
        t_smb = cst.tile([T, BL * BL], f32, tag="t_smb")           # block-diag exp(scoresT)
        t_out = []
        for g in range(G):
            t_out_g = cst.tile([BG, T, V], f32, tag=f"t_out{g}")
            if steps < T:  # truncated builds (timing/sim only): avoid
                nc.vector.memset(t_out_g[:], 0.0)  # uninit reads at final DMA
            t_out.append(t_out_g)
        nc.vector.memset(t_smb[:], 0.0)

        # ---------------- preamble: UaH' = x@Ua + (Ba1+Ba2) ----------------
        NSL, SL = 3, 400  # bt slices per e-chunk
        for ec in range(CH):
            uah_flat = t_uahp[:, ec, :, :].rearrange("p b t -> p (b t)")
            for i in range(NSL):
                ps = prep.tile([NC_, SL], f32, tag="prep")
                for dc in range(CH):
                    x_sl = t_x[:, dc, :, :].rearrange("p b t -> p (b t)")[
                        :, i * SL:(i + 1) * SL]
                    nc.tensor.matmul(ps[:], t_ua[:, dc, ec, :], x_sl,
                                     start=(dc == 0), stop=(dc == CH - 1))
                nc.scalar.activation(uah_flat[:, i * SL:(i + 1) * SL], ps[:],
                                     Act.Identity, bias=t_ba12[:, ec:ec + 1],
                                     scale=1.0)

        # ---------------- preamble: XKC = x@[gruK|Co], ones col ----------------
        for b in range(BL):
            ps = pxm.tile([T, 112], f32, tag="pxm")
            for dc in range(CH):
                nc.tensor.matmul(ps[:], t_x[:, dc, b, :], t_w2[:, dc, :],
                                 start=(dc == 0), stop=(dc == CH - 1))
            nc.vector.tensor_copy(t_xkc[:, b, 0:112], ps[:])
        ones_col = bass.AP(t_xkc.tensor, t_xkc[:].offset + 112,
                           [list(t_xkc[:].ap[0]), [113, BL]])
        nc.vector.memset(ones_col, 1.0)

        # ---------------- recurrent state ----------------
        state = []   # [8,28] b-major per group
        for g in range(G):
            sg = wrk.tile([BG, V], f32, tag=f"state{g}")
            nc.vector.memset(sg[:], 0.0)
            state.append(sg)
        stateT = wrk.tile([V + 1, BL], f32, tag="stateT")  # shared; row V = ones
        nc.vector.memset(stateT[0:V, :], 0.0)
        nc.sync.dma_start(stateT[V:V + 1, :], onesrow[:])

        # ---------------- the 75 steps ----------------
        has_gp_tt = hasattr(nc.gpsimd, "tensor_tensor")
        for s in range(steps):
            new_state = [None] * G
            bt_list = [None] * G
            gorder = (0, 1)
            tanhY = wrk.tile([NC_, CH, BL, T], f32, tag="tanhY")
            # --- phase 0 (shared): WaS^T for both groups ---
            wast_ps = pwast.tile([NC_, 72], f32, tag="wastps")
            if s > 0:
                for c in range(CH):
                    nc.tensor.matmul(wast_ps[:, c * 16:(c + 1) * 16],
                                     t_wa[:, c * 128:(c + 1) * 128],
                                     stateT[0:V, :], start=True, stop=True,
                                     is_transpose=USE_TMODE)
                if has_gp_tt and USE_GP_BIAS:
                    # SBUF copy of the c2/c3 half for GPSIMD (no PSUM there)
                    wast_sb = wrk.tile([NC_, 32], f32, tag="wastsb")
                    nc.vector.tensor_copy(wast_sb[:], wast_ps[:, 32:64])
            # --- phase 1 (per group): state matmuls, bias-add, tanh ---
            for g in gorder:
                sg = state[g]
                sTg = stateT[:, g * BG:(g + 1) * BG]  # [29, 8]; row 28 = 1
                bs = g * BG
                bt_ps = pbt.tile([BG, 256], f32, tag="btps")
                bt_list[g] = bt_ps
                nc.tensor.matmul(bt_ps[:, 0:84], sTg, t_wrec[:],
                                 start=True, stop=True)
                nc.tensor.matmul(bt_ps[:, 84:112], sTg, t_uo[:],
                                 start=True, stop=True)

                ty_out = tanhY[:, :, bs:bs + BG, :]
                if s > 0:
                    Yg = wrk.tile([NC_, CH, BG, T], f32, tag=f"Y{g}")

                    def bias_add(eng, src, off, c0, cn):
                        # WaS^T broadcast over t (src: PSUM for DVE, SBUF copy
                        # for GPSIMD which cannot access PSUM)
                        w_sl = src[:, off + bs:off + bs + 8]
                        w_bc = bass.AP(
                            w_sl.tensor, w_sl.offset,
                            [list(w_sl.ap[0]), [16, cn], [1, 8], [0, T]])
                        eng.tensor_tensor(Yg[:, c0:c0 + cn, :, :],
                                          t_uahp[:, c0:c0 + cn, bs:bs + BG, :],
                                          w_bc, Op.add)
                    # split chunk-pairs so tanh(c01) overlaps bias-add(c23);
                    # c23 on GPSIMD runs concurrently with DVE's c01
                    if has_gp_tt and USE_GP_BIAS:
                        bias_add(nc.gpsimd, wast_sb, 0, 2, 2)
                        bias_add(nc.vector, wast_ps, 0, 0, 2)
                    else:
                        bias_add(nc.vector, wast_ps, 0, 0, 2)
                        bias_add(nc.vector, wast_ps, 32, 2, 2)
                    nc.scalar.activation(tanhY[:, 0:2, bs:bs + BG, :],
                                         Yg[:, 0:2, :, :], Act.Tanh)
                    nc.scalar.activation(tanhY[:, 2:4, bs:bs + BG, :],
                                         Yg[:, 2:4, :, :], Act.Tanh)
                else:
                    nc.scalar.activation(ty_out, t_uahp[:, :, bs:bs + BG, :],
                                         Act.Tanh)

            # --- phase 2 (shared, sub-granular): scoresT -> exp -> SmBlk ->
            # xm/CoC chunks emitted per sub-group so group tails start early
            scT = wrk.tile([T, BL], f32, tag="scT")
            expT = wrk.tile([T, BL], f32, tag="expT")
            xm_list = [pxm.tile([BG, 113], f32, tag="pxm", name=f"xm{g}_{s}")
                       for g in range(G)]
            for b0, nb in SUBS:
                rep_ps = prep.tile([T, 6 * T], f32, tag="prep")
                rep = rep_ps[:].rearrange("p (b t) -> p b t", b=6)[:, 0:nb, :]
                for c in range(CH):
                    nc.tensor.matmul(rep, t_vas[:, c, :],
                                     tanhY[:, c, b0:b0 + nb, :],
                                     start=(c == 0), stop=(c == CH - 1),
                                     is_transpose=USE_TMODE)
                msk = wrk3.tile([T, 6, T], f32, tag="msk")
                d_ap = t_diag[:]
                d_bc = bass.AP(d_ap.tensor, d_ap.offset,
                               [list(d_ap.ap[0]), [0, nb], list(d_ap.ap[1])])
                nc.vector.tensor_tensor(msk[:, 0:nb, :], rep, d_bc, Op.mult)
                nc.vector.tensor_reduce(scT[:, b0:b0 + nb], msk[:, 0:nb, :],
                                        mybir.AxisListType.X, Op.add)
                nc.scalar.activation(expT[:, b0:b0 + nb], scT[:, b0:b0 + nb],
                                     Act.Exp)
                smb_dst = bass.AP(t_smb.tensor, t_smb[:].offset + 17 * b0,
                                  [list(t_smb[:].ap[0]), [17, nb]])
                nc.vector.tensor_copy(smb_dst, expT[:, b0:b0 + nb])
                for b in range(b0, b0 + nb):
                    g = b // BG
                    bs = g * BG
                    nc.tensor.matmul(
                        xm_list[g][:],
                        t_smb[:, 16 * b + bs:16 * b + bs + BG],
                        t_xkc[:, b, :],
                        start=(b == bs), stop=(b == bs + BG - 1))
                for g in range(G):  # groups whose expT slice just completed
                    if b0 < (g + 1) * BG <= b0 + nb:
                        nc.tensor.matmul(bt_list[g][:, 112:113],
                                         expT[:, g * BG:(g + 1) * BG],
                                         t_two[:], start=True, stop=True)

            # --- phase 3 (per group): recip, gates, state, pred ---
            for g in gorder:
                sg = state[g]
                bs = g * BG
                bt_ps = bt_list[g]
                xm_ps = xm_list[g]
                hm_sb = wrk.tile([BG, 84], f32, tag=f"hm{g}")
                nc.vector.tensor_copy(hm_sb[:], bt_ps[:, 0:84])
                rhalf = wrk.tile([BG, 1], f32, tag=f"rhalf{g}")
                nc.vector.reciprocal(rhalf[:], bt_ps[:, 112:113])
                rfull = wrk.tile([BG, 1], f32, tag=f"rfull{g}")
                nc.vector.tensor_scalar(rfull[:], rhalf[:], 2.0, None, Op.mult)

                # --- GRU gates ---
                zr = wrk.tile([BG, 56], f32, tag=f"zr{g}")
                nc.vector.scalar_tensor_tensor(zr[:], xm_ps[:, 0:56], rhalf[:],
                                               hm_sb[:, 0:56], Op.mult, Op.add)
                if gru_b0_nonzero:
                    nc.vector.tensor_tensor(zr[:], zr[:], t_b0[:, 0:56], Op.add)
                tz = wrk.tile([BG, 56], f32, tag=f"tz{g}")
                nc.scalar.activation(tz[:], zr[:], Act.Tanh)
                s1 = wrk.tile([BG, V], f32, tag=f"s1{g}")
                nc.vector.scalar_tensor_tensor(s1[:], tz[:, V:56], 1.0,
                                               hm_sb[:, 56:84], Op.add, Op.mult)
                ah = wrk.tile([BG, V], f32, tag=f"ah{g}")
                nc.vector.scalar_tensor_tensor(ah[:], xm_ps[:, 56:84], rfull[:],
                                               s1[:], Op.mult, Op.add)
                if gru_b0_nonzero:
                    nc.vector.tensor_tensor(ah[:], ah[:], t_b0[:, 56:84], Op.add)
                hh = wrk.tile([BG, V], f32, tag=f"hh{g}")
                nc.scalar.activation(hh[:], ah[:], Act.Tanh)
                d1 = wrk.tile([BG, V], f32, tag=f"d1{g}")
                nc.vector.tensor_sub(d1[:], sg[:], hh[:])
                d2 = wrk.tile([BG, V], f32, tag=f"d2{g}")
                nc.vector.tensor_add(d2[:], sg[:], hh[:])
                m1 = wrk.tile([BG, V], f32, tag=f"m1{g}")
                nc.vector.tensor_mul(m1[:], tz[:, 0:V], d1[:])
                ns = wrk.tile([BG, V], f32, tag=f"state{g}")
                nc.vector.tensor_tensor(ns[:], m1[:], d2[:], Op.add)
                nc.vector.tensor_scalar(ns[:], ns[:], 0.5, None, Op.mult)

                # --- stateT for next step (into shared stateT col-slice) ---
                nc.tensor.transpose(wast_ps[0:V, 64:72], ns[:], t_i16[0:BG, 0:BG])
                nc.vector.tensor_copy(stateT[0:V, bs:bs + BG],
                                      wast_ps[0:V, 64:72])

                # --- pred logits + softmax (Bo+w0 folded into UoH psum) ---
                l1 = wrk.tile([BG, V], f32, tag=f"l1{g}")
                if s > 0:
                    l2 = wrk.tile([BG, V], f32, tag=f"l2{g}")
                    nc.vector.scalar_tensor_tensor(l2[:], t_out[g][:, s - 1, :],
                                                   1.0, t_dw[:], Op.is_ge,
                                                   Op.mult)
                    nc.vector.scalar_tensor_tensor(
                        l1[:], xm_ps[:, 84:112], rfull[:], l2[:],
                        Op.mult, Op.add)
                else:
                    nc.vector.tensor_scalar(l1[:], xm_ps[:, 84:112], rfull[:],
                                            None, Op.mult)
                logits = wrk.tile([BG, V], f32, tag=f"logits{g}")
                nc.vector.tensor_tensor(logits[:], l1[:], bt_ps[:, 84:112],
                                        Op.add)
                expP = wrk.tile([BG, V], f32, tag=f"expP{g}")
                zp = wrk.tile([BG, 1], f32, tag=f"zp{g}")
                nc.scalar.activation(expP[:], logits[:], Act.Exp,
                                     accum_out=zp[:])
                rp = wrk.tile([BG, 1], f32, tag=f"rp{g}")
                nc.vector.reciprocal(rp[:], zp[:])
                nc.vector.tensor_scalar(t_out[g][:, s, :], expP[:], rp[:],
                                        None, Op.mult)

                new_state[g] = ns

            state = new_state

        for g in range(G):
            nc.sync.dma_start(y_out[g * BG:(g + 1) * BG, :, :], t_out[g][:])
    return dr, y_out


_CACHE = {}


def _get_program(gru_b0_nonzero, steps=T):
    key = (bool(gru_b0_nonzero), steps)
    if key in _CACHE:
        return _CACHE[key]
    import concourse.bass as bass
    import concourse.bacc as bacc
    import concourse.tile as tile
    from concourse import mybir

    nc = bacc.Bacc("TRN2", target_bir_lowering=False, debug=False,
                   num_devices=NCORES)
    with tile.TileContext(nc) as tc:
        _build(nc, tc, tile, bass, mybir, gru_b0_nonzero, steps)
    nc.compile()
    _CACHE[key] = nc
    return nc


def _prep_core_inputs(inputs, core):
    x = inputs["x"]
    xs = np.ascontiguousarray(x[core * BL:(core + 1) * BL]).astype(np.float32)
    # [16,75,512] -> [128, chunk, b, t]
    x_dmaj = np.ascontiguousarray(
        xs.reshape(BL, T, CH, 128).transpose(3, 2, 0, 1))
    return x_dmaj


def _prep_weights(inputs):
    f = np.float32
    Ua = inputs["Ua"].astype(f)
    ua_k = np.ascontiguousarray(
        Ua.reshape(CH, 128, CH, 128).transpose(1, 0, 2, 3))
    ba = (inputs["Ba1"] + inputs["Ba2"]).astype(f).reshape(CH, 128)
    ba12 = np.ascontiguousarray(ba.T)
    Va = inputs["Va"].astype(f).reshape(CH, 128)
    vasel = np.ascontiguousarray(
        np.repeat(Va.T[:, :, None], T, axis=2))
    w2 = np.concatenate([inputs["gru_kernel"], inputs["Co"]], axis=1).astype(f)
    w2 = np.ascontiguousarray(w2.reshape(CH, 128, 112).transpose(1, 0, 2))
    w = (inputs["emb"].astype(f) @ inputs["Wo"].astype(f)).reshape(-1)
    w0, w1 = float(w[0]), float(w[1])
    gb = inputs["gru_bias"].astype(f)
    out = {
        "ua_k": ua_k, "ba12": ba12, "wa": inputs["Wa"].astype(f),
        "vasel": vasel, "w2": w2,
        "wrec_h": np.concatenate(
            [0.5 * inputs["gru_rec_kernel"].astype(f), 0.5 * gb[1:2]], axis=0),
        "uo": np.concatenate(
            [inputs["Uo"].astype(f), inputs["Bo"].astype(f) + w0], axis=0),
        "diag": np.eye(T, dtype=f),
        "i16": np.eye(BL, dtype=f),
        "onesrow": np.ones([1, BL], dtype=f),
        "twos75": np.full([T, 1], 2.0, dtype=f),
        "dwrep": np.full([BG, V], w1 - w0, dtype=f),
    }
    b0 = gb[0]
    if np.any(b0 != 0):
        out["b0rep"] = np.repeat(b0[None, :], BG, axis=0)
    return out, bool(np.any(b0 != 0))


def kernel(**inputs):
    from concourse.bass_utils import run_bass_kernel_spmd

    weights, b0nz = _prep_weights(inputs)
    nc = _get_program(b0nz)
    in_maps = []
    for core in range(NCORES):
        m = dict(weights)
        m["x_dmaj"] = _prep_core_inputs(inputs, core)
        in_maps.append(m)
    res = run_bass_kernel_spmd(nc, in_maps, core_ids=list(range(NCORES)))
    out = np.concatenate([res.results[c]["y"] for c in range(NCORES)], axis=0)
    return out.astype(np.float32)





# revision 9
# speedup vs baseline: 1.9818x; 1.6856x over previous
"""Cascaded-attention GRU recurrence on 8 NeuronCores (Bass/Tile), v3.

Problem: B=128, T=75, D=512, V=28. Data-parallel over batch: 16 batch rows
per core, weights replicated. Per-core recurrence over 75 steps with two
INDEPENDENT batch half-groups (8 rows each) running half a step out of
phase so the Activation engine (the per-step floor: tanh over B*T*D) stays
saturated while the other group walks its gate/state-update chain.

Per-core layout (BL=16 local batch, G=2 groups of BG=8):
- UaH' = x@Ua + Ba1 + Ba2 stored bf16 [128(d), 4(chunk), 75(t), 16(b)]
  (t-before-b: the per-step bias broadcast add has stride-1 last dim ->
  DVE 2x mode).
- per step, per group: WaS^T = Wa^T stateT (PE fp32 psum) -> bf16 SBUF;
  Y = UaH' + WaS'(bcast over t), b-half on DVE (bf16 2x) / b-half GPSIMD;
  tanh split in two b-half ACT instrs so the attention matmuls for the
  first b-half start at the half-way point.
- scores vs column-replicated bf16 Va (PE 1cyc/row) -> REP[75,(b,t)] psum;
  diag-mask (DVE) + X-reduce gives scoresT[t, b]; exp (ACT) writes onto
  the block-diagonal of smb[75, 72] (stride-9 AP; zeros persist from
  preamble).
- xm/CoC/Z: 8 accumulated matmuls smb-window^T @ xkc[t,b,113] (bf16);
  col 112 of xkc is ones so xm[:,112] = Z.
- GRU gates: sigmoid via tanh (rec kernel pre-halved on host, 1/(2Z)
  folded into gate scalars). State update computed directly TRANSPOSED:
  nsT = (0.5+0.5 uzT) * sT + (0.5-0.5 uzT) * hhT with uzT/hhT via PE
  transposes, written straight into stateT -- no b-major state tensor.
- output softmax fp32; embedding lookup exact via is_ge (probs cast to
  int32 are 0 unless pred >= 1.0); gru_bias[0] assumed zero (asserted).
"""

import numpy as np

B, T, D, V = 128, 75, 512, 28
NCORES = 8
BL = B // NCORES
G = 2
BG = BL // G
NC_, CH = 128, D // 128
BH = BG // 2            # b-half within a group (tanh/bias/score split)


def _build(nc, tc, tile, bass, mybir, gru_b0_nonzero, steps=T):
    f32 = mybir.dt.float32
    bf16 = mybir.dt.bfloat16
    Act = mybir.ActivationFunctionType
    Op = mybir.AluOpType
    AP = bass.AP

    dr = {}
    def din(name, shape, dt=f32):
        dr[name] = nc.dram_tensor(name, shape, dt, kind="ExternalInput")
        return dr[name]

    x_b = din("x_b", [NC_, CH, BL, T], bf16)
    ua_k = din("ua_k", [NC_, CH, CH, 128], bf16)
    ba12 = din("ba12", [NC_, CH])
    wa = din("wa", [V, D])
    vasel = din("vasel", [NC_, CH, T], bf16)
    w2 = din("w2", [NC_, CH, 112], bf16)
    wrec_h = din("wrec_h", [V + 1, 84])
    uo = din("uo", [V + 1, V])
    diag = din("diag", [T, T], bf16)
    i8 = din("i8", [BG, BG])
    onesrow = din("onesrow", [1, BL])
    dwrep = din("dwrep", [BG, V])
    if gru_b0_nonzero:
        b0rep = din("b0rep", [BG, 84])
    y_out = nc.dram_tensor("y", [BL, T, V], f32, kind="ExternalOutput")

    import contextlib
    ctx = contextlib.ExitStack()
    with ctx:
        cst = ctx.enter_context(tc.tile_pool(name="cst", bufs=1))
        wrk = ctx.enter_context(tc.tile_pool(name="wrk", bufs=2))
        pwast = ctx.enter_context(tc.tile_pool(name="pwast", bufs=2, space="PSUM"))
        prep = ctx.enter_context(tc.tile_pool(name="prep", bufs=2, space="PSUM"))
        pxm = ctx.enter_context(tc.tile_pool(name="pxm", bufs=3, space="PSUM"))

        # ---------------- constants ----------------
        t_x = cst.tile([NC_, CH, BL, T], bf16, tag="t_x")
        t_ua = cst.tile([NC_, CH, CH, 128], bf16, tag="t_ua")
        t_ba12 = cst.tile([NC_, CH], f32, tag="t_ba12")
        t_wa = cst.tile([V, D], f32, tag="t_wa")
        t_vas = cst.tile([NC_, CH, T], bf16, tag="t_vas")
        t_w2 = cst.tile([NC_, CH, 112], bf16, tag="t_w2")
        t_wrec = cst.tile([V + 1, 84], f32, tag="t_wrec")
        t_uo = cst.tile([V + 1, V], f32, tag="t_uo")
        t_diag = cst.tile([T, T], bf16, tag="t_diag")
        t_i8 = cst.tile([BG, BG], f32, tag="t_i8")
        t_dw = cst.tile([BG, V], f32, tag="t_dw")
        for tt, d_ in [(t_x, x_b), (t_ua, ua_k), (t_ba12, ba12), (t_wa, wa),
                       (t_vas, vasel), (t_w2, w2), (t_wrec, wrec_h),
                       (t_uo, uo), (t_diag, diag), (t_i8, i8), (t_dw, dwrep)]:
            nc.sync.dma_start(tt[:], d_[:])
        if gru_b0_nonzero:
            t_b0 = cst.tile([BG, 84], f32, tag="t_b0")
            nc.sync.dma_start(t_b0[:], b0rep[:])

        t_uahp = cst.tile([NC_, CH, T, BL], bf16, tag="t_uahp")
        t_xkc = cst.tile([T, BL, 113], bf16, tag="t_xkc")
        t_tanh = [cst.tile([NC_, CH, T, BG], bf16, tag=f"t_tanh{g}",
                           name=f"t_tanh{g}") for g in range(G)]
        t_smb = [cst.tile([T, BG * (BG + 1)], bf16, tag=f"t_smb{g}",
                          name=f"t_smb{g}") for g in range(G)]
        t_wasb = [cst.tile([NC_, CH, BG], bf16, tag=f"t_wasb{g}",
                           name=f"t_wasb{g}") for g in range(G)]
        t_stT = [cst.tile([V + 1, BG], f32, tag=f"t_stT{g}",
                          name=f"t_stT{g}") for g in range(G)]
        t_out = []
        for g in range(G):
            t_out_g = cst.tile([BG, T, V], f32, tag=f"t_out{g}")
            if steps < T:
                nc.vector.memset(t_out_g[:], 0.0)
            t_out.append(t_out_g)
            nc.vector.memset(t_smb[g][:], 0.0)
            nc.vector.memset(t_stT[g][0:V, :], 0.0)
            nc.sync.dma_start(t_stT[g][V:V + 1, :], onesrow[:, g * BG:(g + 1) * BG])

        # ---------------- preamble: UaH' ----------------
        NSL, SLB = 4, 4
        for ec in range(CH):
            for i in range(NSL):
                b0 = i * SLB
                ps = prep.tile([NC_, T * SLB], f32, tag="rep",
                               name=f"preu{ec}_{i}")
                psv = ps[:].rearrange("p (t b) -> p t b", t=T)
                for dc in range(CH):
                    xs = t_x[:, dc, :, :]
                    x_sl = AP(xs.tensor, xs.offset + b0 * T,
                              [list(xs.ap[0]), [1, T], [T, SLB]])
                    nc.tensor.matmul(psv, t_ua[:, dc, ec, :], x_sl,
                                     start=(dc == 0), stop=(dc == CH - 1))
                dst = t_uahp[:, ec, :, b0:b0 + SLB]
                if i % 2 == 0:
                    nc.scalar.activation(dst, psv, Act.Identity,
                                         bias=t_ba12[:, ec:ec + 1], scale=1.0)
                else:
                    nc.vector.tensor_scalar(dst, psv, t_ba12[:, ec:ec + 1],
                                            None, Op.add)

        # ---------------- preamble: XKC ----------------
        for b in range(BL):
            ps = pxm.tile([T, 113], f32, tag="xmbt", name=f"prex{b}")
            for dc in range(CH):
                nc.tensor.matmul(ps[:, 0:112], t_x[:, dc, b, :], t_w2[:, dc, :],
                                 start=(dc == 0), stop=(dc == CH - 1))
            if b % 2 == 0:
                nc.scalar.activation(t_xkc[:, b, 0:112], ps[:, 0:112],
                                     Act.Identity)
            else:
                nc.vector.tensor_copy(t_xkc[:, b, 0:112], ps[:, 0:112])
        ones_col = AP(t_xkc.tensor, t_xkc[:].offset + 112,
                      [list(t_xkc[:].ap[0]), [113, BL]])
        nc.vector.memset(ones_col, 1.0)

        P = {}

        def emit_bt(g, s):
            """hm/uo matmuls into the xmbt psum tile (cols 128:240); reads
            stateT for step s (already updated)."""
            xmbt = pxm.tile([BG, 240], f32, tag="xmbt", name=f"xmbt{g}_{s}")
            P[(g, s, "xmbt")] = xmbt
            nc.tensor.matmul(xmbt[:, 128:212], t_stT[g][:], t_wrec[:],
                             start=True, stop=True)
            nc.tensor.matmul(xmbt[:, 212:240], t_stT[g][:], t_uo[:],
                             start=True, stop=True)

        def phase_TH(g, s, half):
            """tanh b-half on ACT (bf16 out)."""
            b0 = half * BH
            if s == 0:
                src = t_uahp[:, :, :, g * BG + b0:g * BG + b0 + BH]
            else:
                src = P[(g, s, "Y")][:, :, :, b0:b0 + BH]
            nc.scalar.activation(t_tanh[g][:, :, :, b0:b0 + BH], src, Act.Tanh)

        def phase_SC(g, s):
            """scores/diag-extract per b-half, exp, xm accumulation, rZ."""
            tg = t_tanh[g]
            scT = wrk.tile([T, BG], bf16, tag=f"scT{g}", name=f"scT{g}_{s}")
            for half in range(2):
                b0 = half * BH
                rep = prep.tile([NC_, T * BH], f32, tag="rep",
                                name=f"rep{g}_{s}_{b0}")
                rp = rep[0:T, :].rearrange("p (b t) -> p b t", b=BH)
                for c in range(CH):
                    tc_ = tg[:, c, :, :]
                    mv = AP(tc_.tensor, tc_.offset + b0,
                            [list(tc_.ap[0]), [1, BH], [BG, T]])
                    nc.tensor.matmul(rp, t_vas[:, c, :], mv,
                                     start=(c == 0), stop=(c == CH - 1))
                msk = wrk.tile([T, BH, T], bf16, tag=f"msk{g}",
                               name=f"msk{g}_{s}_{b0}")
                d_ap = t_diag[:]
                d_bc = AP(d_ap.tensor, d_ap.offset,
                          [list(d_ap.ap[0]), [0, BH], list(d_ap.ap[1])])
                nc.vector.tensor_tensor(msk[:], rp, d_bc, Op.mult)
                with nc.allow_low_precision("diag extract: 1 nonzero"):
                    nc.vector.tensor_reduce(scT[:, b0:b0 + BH], msk[:],
                                            mybir.AxisListType.X, Op.add)
            smb = t_smb[g]
            smb_diag = AP(smb.tensor, smb[:].offset,
                          [list(smb[:].ap[0]), [BG + 1, BG]])
            nc.scalar.activation(smb_diag, scT[:], Act.Exp)
            xm = P[(g, s, "xmbt")][:, 0:113]
            for b in range(BG):
                nc.tensor.matmul(xm, smb[:, BG * b:BG * b + BG],
                                 t_xkc[:, g * BG + b, :],
                                 start=(b == 0), stop=(b == BG - 1))
            rZ = wrk.tile([BG, 1], f32, tag=f"rZ{g}", name=f"rZ{g}_{s}")
            nc.vector.reciprocal(rZ[:], xm[:, 112:113])
            P[(g, s, "rZ")] = rZ
            rh = wrk.tile([BG, 1], f32, tag=f"rh{g}", name=f"rh{g}_{s}")
            nc.vector.tensor_scalar(rh[:], rZ[:], 0.5, None, Op.mult)
            P[(g, s, "rh")] = rh

        def phase_G1(g, s):
            """zr (DVE) -> tz (ACT)."""
            xmbt = P[(g, s, "xmbt")]
            zr = wrk.tile([BG, 56], f32, tag=f"zr{g}", name=f"zr{g}_{s}")
            nc.vector.scalar_tensor_tensor(zr[:], xmbt[:, 0:56],
                                           P[(g, s, "rh")][:],
                                           xmbt[:, 128:184], Op.mult, Op.add)
            if gru_b0_nonzero:
                nc.vector.tensor_tensor(zr[:], zr[:], t_b0[:, 0:56], Op.add)
            tz = wrk.tile([BG, 56], f32, tag=f"tz{g}", name=f"tz{g}_{s}")
            nc.scalar.activation(tz[:], zr[:], Act.Tanh)
            P[(g, s, "tz")] = tz

        def phase_G2(g, s):
            """s1/ah -> hh; transposed state update into stateT; then WaS,
            wasb, bias and bt for step s+1."""
            xmbt = P[(g, s, "xmbt")]
            tz = P[(g, s, "tz")]
            wast = pwast.tile([NC_, 48], f32, tag="wast", name=f"wast{g}_{s}")
            # on-chain: s1, ah (DVE), hh (ACT)
            s1 = wrk.tile([BG, V], f32, tag=f"s1{g}", name=f"s1{g}_{s}")
            nc.vector.scalar_tensor_tensor(s1[:], tz[:, V:56], 1.0,
                                           xmbt[:, 184:212], Op.add, Op.mult)
            ah = wrk.tile([BG, V], f32, tag=f"ah{g}", name=f"ah{g}_{s}")
            nc.vector.scalar_tensor_tensor(ah[:], xmbt[:, 56:84],
                                           P[(g, s, "rZ")][:], s1[:],
                                           Op.mult, Op.add)
            if gru_b0_nonzero:
                nc.vector.tensor_tensor(ah[:], ah[:], t_b0[:, 56:84], Op.add)
            hh = wrk.tile([BG, V], f32, tag=f"hh{g}", name=f"hh{g}_{s}")
            nc.scalar.activation(hh[:], ah[:], Act.Tanh)
            # off-chain: transpose tz_z, build Z0T/W0T/M1T
            nc.tensor.transpose(wast[0:V, 32:40], tz[:, 0:V], t_i8[:])
            z0 = wrk.tile([V, BG], f32, tag=f"z0{g}", name=f"z0{g}_{s}")
            nc.vector.tensor_scalar(z0[:], wast[0:V, 32:40], 0.5, 0.5,
                                    Op.mult, Op.add)
            w0 = wrk.tile([V, BG], f32, tag=f"w0{g}", name=f"w0{g}_{s}")
            nc.vector.tensor_scalar(w0[:], wast[0:V, 32:40], -0.5, 0.5,
                                    Op.mult, Op.add)
            m1 = wrk.tile([V, BG], f32, tag=f"m1{g}", name=f"m1{g}_{s}")
            nc.vector.tensor_tensor(m1[:], z0[:], t_stT[g][0:V, :], Op.mult)
            # on-chain: hhT, nsT -> stateT
            nc.tensor.transpose(wast[0:V, 40:48], hh[:], t_i8[:])
            n1 = wrk.tile([V, BG], f32, tag=f"n1{g}", name=f"n1{g}_{s}")
            nc.vector.tensor_tensor(n1[:], w0[:], wast[0:V, 40:48], Op.mult)
            nc.vector.tensor_tensor(t_stT[g][0:V, :], n1[:], m1[:], Op.add)
            if s + 1 >= steps:
                return
            # WaS for s+1
            for c in range(CH):
                nc.tensor.matmul(wast[:, c * BG:(c + 1) * BG],
                                 t_wa[:, c * 128:(c + 1) * 128],
                                 t_stT[g][0:V, :], start=True, stop=True)
            emit_bt(g, s + 1)
            wb = t_wasb[g]
            nc.vector.tensor_copy(wb[:].rearrange("p c b -> p (c b)"),
                                  wast[:, 0:CH * BG])
            ty = P[(g, s + 1, "Y")] = wrk.tile([NC_, CH, T, BG], bf16,
                                               tag=f"Y{g}", name=f"Y{g}_{s+1}")
            for (eng, b0) in ((nc.vector, 0), (nc.gpsimd, BH)):
                w_sl = AP(wb.tensor, wb[:].offset + b0,
                          [list(wb[:].ap[0]), [BG, CH], [0, T], [1, BH]])
                eng.tensor_tensor(
                    ty[:, :, :, b0:b0 + BH],
                    t_uahp[:, :, :, g * BG + b0:g * BG + b0 + BH],
                    w_sl, Op.add)

        def phase_E(g, s):
            """output softmax; off the recurrence chain."""
            xmbt = P.pop((g, s, "xmbt"))
            rZ = P.pop((g, s, "rZ"))
            P.pop((g, s, "rh"), None)
            P.pop((g, s, "tz"), None)
            P.pop((g, s, "Y"), None)
            l1 = wrk.tile([BG, V], f32, tag=f"l1{g}", name=f"l1{g}_{s}")
            if s > 0:
                l2 = wrk.tile([BG, V], f32, tag=f"l2{g}", name=f"l2{g}_{s}")
                nc.gpsimd.scalar_tensor_tensor(l2[:], t_out[g][:, s - 1, :],
                                               1.0, t_dw[:], Op.is_ge,
                                               Op.mult)
                nc.vector.scalar_tensor_tensor(l1[:], xmbt[:, 84:112], rZ[:],
                                               l2[:], Op.mult, Op.add)
            else:
                nc.vector.tensor_scalar(l1[:], xmbt[:, 84:112], rZ[:], None,
                                        Op.mult)
            logits = wrk.tile([BG, V], f32, tag=f"lg{g}", name=f"lg{g}_{s}")
            nc.vector.tensor_tensor(logits[:], l1[:], xmbt[:, 212:240], Op.add)
            expP = wrk.tile([BG, V], f32, tag=f"eP{g}", name=f"eP{g}_{s}")
            nc.scalar.activation(expP[:], logits[:], Act.Exp)
            zp = wrk.tile([BG, 1], f32, tag=f"zp{g}", name=f"zp{g}_{s}")
            nc.vector.tensor_reduce(zp[:], expP[:], mybir.AxisListType.X,
                                    Op.add)
            rp = wrk.tile([BG, 1], f32, tag=f"rp{g}", name=f"rp{g}_{s}")
            nc.vector.reciprocal(rp[:], zp[:])
            nc.gpsimd.tensor_scalar(t_out[g][:, s, :], expP[:], rp[:], None,
                                    Op.mult)

        # bt for step 0 (stateT is zeros+ones)
        emit_bt(0, 0)
        emit_bt(1, 0)

        # ---------------- steps: two groups, half-step offset ---------------
        for s in range(steps):
            phase_TH(0, s, 0)
            if s > 0:
                phase_G1(1, s - 1)
            phase_TH(0, s, 1)
            if s > 0:
                phase_G2(1, s - 1)
            phase_SC(0, s)
            if s > 0:
                phase_E(1, s - 1)
            phase_TH(1, s, 0)
            if s + 1 < steps:
                phase_G1(0, s)
            phase_TH(1, s, 1)
            if s + 1 < steps:
                phase_G2(0, s)
            phase_SC(1, s)
            phase_E(0, s)
        phase_E(1, steps - 1)

        for g in range(G):
            nc.sync.dma_start(y_out[g * BG:(g + 1) * BG, :, :], t_out[g][:])
    return dr, y_out


_CACHE = {}


def _get_program(gru_b0_nonzero, steps=T):
    key = (bool(gru_b0_nonzero), steps)
    if key in _CACHE:
        return _CACHE[key]
    import concourse.bass as bass
    import concourse.bacc as bacc
    import concourse.tile as tile
    from concourse import mybir

    nc = bacc.Bacc("TRN2", target_bir_lowering=False, debug=False,
                   num_devices=NCORES)
    with tile.TileContext(nc) as tc:
        _build(nc, tc, tile, bass, mybir, gru_b0_nonzero, steps)
    nc.compile()
    _CACHE[key] = nc
    return nc


def _prep_core_inputs(inputs, core):
    import ml_dtypes
    x = inputs["x"]
    xs = np.ascontiguousarray(x[core * BL:(core + 1) * BL]).astype(np.float32)
    x_dmaj = np.ascontiguousarray(
        xs.reshape(BL, T, CH, 128).transpose(3, 2, 0, 1))
    return x_dmaj.astype(ml_dtypes.bfloat16)


def _prep_weights(inputs):
    import ml_dtypes
    f = np.float32
    bfd = ml_dtypes.bfloat16
    Ua = inputs["Ua"].astype(f)
    ua_k = np.ascontiguousarray(
        Ua.reshape(CH, 128, CH, 128).transpose(1, 0, 2, 3))
    ba = (inputs["Ba1"] + inputs["Ba2"]).astype(f).reshape(CH, 128)
    ba12 = np.ascontiguousarray(ba.T)
    Va = inputs["Va"].astype(f).reshape(CH, 128)
    vasel = np.ascontiguousarray(np.repeat(Va.T[:, :, None], T, axis=2))
    w2 = np.concatenate([inputs["gru_kernel"], inputs["Co"]], axis=1).astype(f)
    w2 = np.ascontiguousarray(w2.reshape(CH, 128, 112).transpose(1, 0, 2))
    w = (inputs["emb"].astype(f) @ inputs["Wo"].astype(f)).reshape(-1)
    w0, w1 = float(w[0]), float(w[1])
    gb = inputs["gru_bias"].astype(f)
    out = {
        "ua_k": ua_k.astype(bfd), "ba12": ba12,
        "wa": inputs["Wa"].astype(f),
        "vasel": vasel.astype(bfd), "w2": w2.astype(bfd),
        "wrec_h": np.concatenate(
            [0.5 * inputs["gru_rec_kernel"].astype(f), 0.5 * gb[1:2]], axis=0),
        "uo": np.concatenate(
            [inputs["Uo"].astype(f), inputs["Bo"].astype(f) + w0], axis=0),
        "diag": np.eye(T, dtype=f).astype(bfd),
        "i8": np.eye(BG, dtype=f),
        "onesrow": np.ones([1, BL], dtype=f),
        "dwrep": np.full([BG, V], w1 - w0, dtype=f),
    }
    b0 = gb[0]
    if np.any(b0 != 0):
        out["b0rep"] = np.repeat(b0[None, :], BG, axis=0)
    return out, bool(np.any(b0 != 0))


def kernel(**inputs):
    from concourse.bass_utils import run_bass_kernel_spmd

    weights, b0nz = _prep_weights(inputs)
    nc = _get_program(b0nz)
    in_maps = []
    for core in range(NCORES):
        m = dict(weights)
        m["x_b"] = _prep_core_inputs(inputs, core)
        in_maps.append(m)
    res = run_bass_kernel_spmd(nc, in_maps, core_ids=list(range(NCORES)))
    out = np.concatenate([res.results[c]["y"] for c in range(NCORES)], axis=0)
    return out.astype(np.float32)
